# revision 38
# baseline (speedup 1.0000x reference)
"""Trainium2 Bass kernel for nn_CausalMultimodal (gnn_message_passing).

Math (per batch row b, fully row-local so batch shards freely over 8 cores):
    mask[i,j]  = (matrix*(matrix>0.1))[i,j] > 0.1
    agg[b,d]   = (Z[b,:] @ mask[d,:]) / count[d]   (0 when count==0)
    hidden     = relu(Z[b,d]*W1[d,0,h] + agg[b,d]*W1[d,1,h] + b1[d,h])
    E[b,d]     = sum_h hidden[b,d,h]*W2[d,h] + b2[d]

Since agg = Z @ M2 with M2[j,d] = mask[d,j]/count[d], the whole first layer
folds into one 32x128 matrix A computed host-side from the tiny params:
U[b, 32h+d] = (Z @ A)[b, 32h+d]; then E = W2sel.T @ relu(U + b1) + b2 with
W2sel (128,32) block-sparse.

v4 dataflow: ALL transposes are done host-side (numpy), so the device runs
only matmuls + relu + PSUM->SBUF copies and contiguous DMA:
  host: Zt[i, 32q+j, c] = Z[i*4F + q*F + c, j]  (bf16)
  per (128, F) megatile: DMA in -> per 512-col block: 4x mm1 (row bands,
  K=32) into a 3-bank (128,1536) + 1-bank (128,512) PSUM pair -> one wide
  relu (1536) on one engine + relu3 (512) on the other -> 4x mm3 (col
  bands, K=128) into eps (row 32q+d = E[d] of band-q batches) -> copy
  PSUM->SBUF bf16 -> DMA out; host unpermutes E'[i,32q+d,c] ->
  E[i*4F+q*F+c, d].  ACT/DVE roles alternate per block to balance load;
  PSUM: ug012 x2 (6 banks) + ug3 (1) + eps (1) = 8.
"""

import os

import ml_dtypes
import numpy as np

import concourse.bacc as bacc
import concourse.tile as tile
from concourse import mybir
from concourse import bass_utils

B_TOTAL, D, H = 1048576, 32, 4
NCORES = 8
R = B_TOTAL // NCORES  # rows per core
BF16 = ml_dtypes.bfloat16

F = int(os.environ.get("NNK_F", "4096"))  # main megatile free size (cols)
OUT_DT = os.environ.get("NNK_OUT", "bf16")  # bf16 | f32 output E dtype
WARM = int(os.environ.get("NNK_WARM", "5"))  # p-state warmup matmuls

# per-megatile free sizes; 4*sum(MTS) == R (rows per core).  Ramp-up and
# ramp-down ladders keep the pipeline fill (first loads) and drain (last
# stores) short while the middle uses big tiles.
_head, _tail = [1024, 2048], [2048, 2048, 1024]
_mid_total = R // 4 - sum(_head) - sum(_tail)
_mid = [F] * (_mid_total // F)
if _mid_total % F:
    _mid.append(_mid_total % F)
MTS = _head + _mid + _tail
assert 4 * sum(MTS) == R and all(m % 512 == 0 and m > 0 for m in MTS)
TOTC = sum(MTS)  # total columns per core

_module_cache = {}


def _build_module(rows, b2_zero):
    key = (rows, b2_zero, tuple(MTS), OUT_DT, WARM)
    if key in _module_cache:
        return _module_cache[key]

    f32 = mybir.dt.float32
    bf = mybir.dt.bfloat16
    odt = bf if OUT_DT == "bf16" else f32

    nc = bacc.Bacc("TRN2", target_bir_lowering=False, debug=False,
                   num_devices=NCORES)

    nmt = len(MTS)
    fmax = max(MTS)

    ZT = nc.dram_tensor("ZT", (128, TOTC), bf, kind="ExternalInput").ap()
    A4 = nc.dram_tensor("A4", (128, 128), bf, kind="ExternalInput").ap()
    W2S = nc.dram_tensor("W2S", (128, D), bf, kind="ExternalInput").ap()
    B1V = nc.dram_tensor("B1V", (128, 1), f32, kind="ExternalInput").ap()
    B2V = nc.dram_tensor("B2V", (128, 1), f32, kind="ExternalInput").ap()
    EP = nc.dram_tensor("EP", (128, TOTC), odt, kind="ExternalOutput").ap()

    def relu_op(eng, vg, ug, b1v):
        if eng == "A":
            nc.scalar.activation(vg, ug, mybir.ActivationFunctionType.Relu,
                                 bias=b1v, scale=1.0)
        else:
            nc.vector.tensor_scalar(vg, ug, b1v, 0.0,
                                    mybir.AluOpType.add, mybir.AluOpType.max)

    def copy_op(eng, out, in_, b2v):
        if not b2_zero or eng == "A":
            nc.scalar.activation(out, in_,
                                 mybir.ActivationFunctionType.Identity,
                                 bias=(0.0 if b2_zero else b2v), scale=1.0)
        else:
            nc.vector.tensor_copy(out, in_)

    with tile.TileContext(nc) as tc:
        with (
            tc.tile_pool(name="const", bufs=1) as constp,
            tc.tile_pool(name="zt", bufs=2) as ztp,
            tc.tile_pool(name="vv", bufs=2) as vp,
            tc.tile_pool(name="en", bufs=2) as enp,
            tc.tile_pool(name="pu", bufs=1, space="PSUM") as pup,
            tc.tile_pool(name="pe", bufs=1, space="PSUM") as pep,
        ):
            # DMA order matters on the single input ring: A4 (needed by the
            # first mm1) and the first Z tile go first; the remaining
            # consts are only needed once relu/mm3 start.
            acst = constp.tile([128, 128], bf, name="cA4")
            nc.sync.dma_start(out=acst, in_=A4)
            fmax = max(MTS)
            zt0 = ztp.tile([128, fmax], bf, tag="zt")
            # first 512 cols (all block 0 needs) on the SP ring right after
            # A4; the rest rides the idle Activation ring in parallel.  The
            # dispatches have no upstream deps, so no head-of-line risk.
            nc.sync.dma_start(out=zt0[:, :512], in_=ZT[:, :512])
            nc.scalar.dma_start(out=zt0[:, 512:MTS[0]],
                                in_=ZT[:, 512:MTS[0]])
            wcst = constp.tile([128, D], bf, name="cW2S")
            nc.sync.dma_start(out=wcst, in_=W2S)
            b1v = constp.tile([128, 1], f32)
            nc.sync.dma_start(out=b1v, in_=B1V)
            b2v = constp.tile([128, 1], f32)
            nc.sync.dma_start(out=b2v, in_=B2V)

            if WARM:
                # dummy matmuls during the initial DMA wait: keeps the PE
                # busy so the p-state ramp to full clock completes before
                # real work arrives.  Reads a memset scratch tile (no DMA
                # dependency); writes land in the eps bank and are
                # overwritten by block 0.
                wtile = constp.tile([128, 512], bf, name="warm")
                nc.gpsimd.memset(wtile, 0.0)
                epw = pep.tile([128, 512], f32, bufs=1, tag="eps",
                               name="epw")
                for _ in range(WARM):
                    nc.tensor.matmul(epw, lhsT=wtile[:, 0:128], rhs=wtile,
                                     start=True, stop=True,
                                     tile_position=(0, 0))

            off = 0
            for i, fi in enumerate(MTS):
                if i == 0:
                    zt = zt0
                else:
                    zt = ztp.tile([128, fmax], bf, tag="zt")
                    nc.sync.dma_start(out=zt[:, :fi],
                                      in_=ZT[:, off:off + fi])
                enat = enp.tile([128, fmax], odt, tag="en")
                for t in range(fi // 512):
                    sl = slice(512 * t, 512 * (t + 1))
                    # PSUM banks: ug012 (3) x2 bufs + ug3 (1) + eps (1) = 8.
                    ug012 = pup.tile([128, 1536], f32, tag="ug012",
                                     name="ug012", bufs=2)
                    ug3 = pup.tile([128, 512], f32, tag="ug3", name="ug3",
                                   bufs=1)
                    for q in range(4):
                        nc.tensor.matmul(
                            ug012[:, 512 * q:512 * (q + 1)] if q < 3 else ug3,
                            lhsT=acst[32 * q:32 * (q + 1), :],
                            rhs=zt[32 * q:32 * (q + 1), sl],
                            start=True, stop=True,
                            tile_position=(32 * q, 0),
                        )
                    # alternate ACT/DVE roles per block: one engine does the
                    # wide relu012, the other does relu3 (which gates the
                    # single-buffered ug3) + the eps copy.  Alternation keeps
                    # consecutive blocks' critical relus on different
                    # engines (no head-of-line blocking).
                    e_wide, e_rest = ("A", "D") if t % 2 == 0 else ("D", "A")
                    vg012 = vp.tile([128, 1536], bf, tag="vg012",
                                    name="vg012")
                    vg3 = vp.tile([128, 512], bf, tag="vg3", name="vg3")
                    relu_op(e_rest, vg3, ug3, b1v)
                    relu_op(e_wide, vg012, ug012, b1v)
                    eps = pep.tile([128, 512], f32, bufs=1, tag="eps")
                    for q in range(4):
                        nc.tensor.matmul(
                            eps[32 * q:32 * (q + 1), :],
                            lhsT=wcst,
                            rhs=vg012[:, 512 * q:512 * (q + 1)]
                            if q < 3 else vg3,
                            start=True, stop=True,
                            tile_position=(0, 32 * q),
                        )
                    # the eps copy goes on e_wide(t): its next job,
                    # relu3(t+1), only becomes ready after mm3(t) anyway,
                    # so the copy's wait on mm3(t) blocks nothing --
                    # whereas on e_rest(t) it would delay relu012(t+1).
                    copy_op(e_wide, enat[:, sl], eps, b2v)
                    if i >= nmt - 2:
                        # chunked trailing stores so the final DMAs drain
                        # early instead of queueing a late 1MB transfer
                        osl = slice(off + 512 * t, off + 512 * (t + 1))
                        nc.sync.dma_start(out=EP[:, osl], in_=enat[:, sl])
                if i < nmt - 2:
                    nc.sync.dma_start(out=EP[:, off:off + fi],
                                      in_=enat[:, :fi])
                off += fi

    nc.compile()
    _module_cache[key] = nc
    return nc


def _fold_params(matrix, W1, b1, W2, b2):
    """Host-side fold of the tiny params into A4/W2S/B1V/B2V (a few KB)."""
    matrix = np.asarray(matrix, np.float32)
    W1 = np.asarray(W1, np.float32)
    b1 = np.asarray(b1, np.float32)
    W2 = np.asarray(W2, np.float32)
    b2 = np.asarray(b2, np.float32)

    alpha_est = matrix * (matrix > np.float32(0.1)).astype(np.float32)
    mask = (alpha_est > np.float32(0.1)).astype(np.float32)  # (D, D)
    cnt = mask.sum(axis=1)  # (D,)
    scale = np.where(cnt > 0, np.float32(1.0) / np.maximum(cnt, 1.0),
                     np.float32(0.0)).astype(np.float32)
    M2 = (mask.T * scale[None, :]).astype(np.float32)  # M2[j,d]

    A = np.zeros((D, D * H), np.float32)
    for h in range(H):
        Ah = M2 * W1[None, :, 1, h]  # (j, d): M2[j,d] * W1[d,1,h]
        Ah[np.arange(D), np.arange(D)] += W1[:, 0, h]
        A[:, D * h:D * (h + 1)] = Ah
    A4 = np.ascontiguousarray(np.tile(A, (4, 1)))  # (128, 128)

    W2S = np.zeros((D * H, D), np.float32)
    W2S[np.arange(D * H), np.tile(np.arange(D), H)] = W2.T.reshape(-1)
    B1V = np.ascontiguousarray(b1.T.reshape(D * H, 1))
    B2V = np.ascontiguousarray(np.tile(b2, H).reshape(D * H, 1))
    b2_zero = not np.any(b2)
    return A4, W2S, B1V, B2V, b2_zero


def _prep_z(Z):
    """Per megatile i (free size fi at col offset oi):
    Zt[c][32q+j, oi+col] = Z[c*R + 4*oi + q*fi + col, j] in bf16."""
    Zr = np.asarray(Z, np.float32).astype(BF16).reshape(NCORES, R, D)
    out = np.empty((NCORES, 128, TOTC), BF16)
    for c in range(NCORES):
        off = 0
        for fi in MTS:
            blk = Zr[c, 4 * off:4 * (off + fi)]    # (4*fi, D)
            blk = blk.reshape(4, fi, D).transpose(0, 2, 1)  # q, j, col
            out[c, :, off:off + fi] = blk.reshape(128, fi)
            off += fi
    return out


def _unpermute_e(chunks):
    """Invert: EP[32q+d, oi+col] = E[4*oi + q*fi + col, d] per core."""
    es = []
    for ep in chunks:
        e = np.empty((R, D), ep.dtype)
        off = 0
        for fi in MTS:
            blk = ep[:, off:off + fi].reshape(4, D, fi)
            e[4 * off:4 * (off + fi)] = (
                blk.transpose(0, 2, 1).reshape(4 * fi, D))
            off += fi
        es.append(e)
    out = np.concatenate(es, axis=0)
    if out.dtype != np.float32:
        out = out.astype(np.float32)
    return out


def _run(Z, matrix, W1, b1, W2, b2, trace=False):
    assert np.asarray(Z).shape == (B_TOTAL, D)
    A4, W2S, B1V, B2V, b2_zero = _fold_params(matrix, W1, b1, W2, b2)
    nc = _build_module(R, b2_zero)

    Zt = _prep_z(Z)
    cst = {
        "A4": np.ascontiguousarray(A4.astype(BF16)),
        "W2S": np.ascontiguousarray(W2S.astype(BF16)),
        "B1V": B1V, "B2V": B2V,
    }
    in_maps = [{**cst, "ZT": np.ascontiguousarray(Zt[c])}
               for c in range(NCORES)]
    res = bass_utils.run_bass_kernel_spmd(
        nc, in_maps, core_ids=list(range(NCORES)), trace=trace)
    out = _unpermute_e([r["EP"] for r in res.results])
    return out, res


def kernel(Z, matrix, W1, b1, W2, b2):
    out, _ = _run(Z, matrix, W1, b1, W2, b2, trace=False)
    return out


# revision 39
# speedup vs baseline: 1.1450x; 1.1450x over previous
"""Trainium2 Bass kernel for nn_CausalMultimodal (gnn_message_passing).

Math (per batch row b, fully row-local so batch shards freely over 8 cores):
    mask[i,j]  = (matrix*(matrix>0.1))[i,j] > 0.1
    agg[b,d]   = (Z[b,:] @ mask[d,:]) / count[d]   (0 when count==0)
    hidden     = relu(Z[b,d]*W1[d,0,h] + agg[b,d]*W1[d,1,h] + b1[d,h])
    E[b,d]     = sum_h hidden[b,d,h]*W2[d,h] + b2[d]

Since agg = Z @ M2 with M2[j,d] = mask[d,j]/count[d], the whole first layer
folds into one 32x128 matrix A computed host-side from the tiny params:
U[b, 32h+d] = (Z @ A)[b, 32h+d]; then E = W2sel.T @ relu(U + b1) + b2 with
W2sel (128,32) block-sparse.

v4 dataflow: ALL transposes are done host-side (numpy), so the device runs
only matmuls + relu + PSUM->SBUF copies and contiguous DMA:
  host: Zt[i, 32q+j, c] = Z[i*4F + q*F + c, j]  (bf16)
  per (128, F) megatile: DMA in -> per 512-col block: 4x mm1 (row bands,
  K=32) into a 3-bank (128,1536) + 1-bank (128,512) PSUM pair -> one wide
  relu (1536) on one engine + relu3 (512) on the other -> 4x mm3 (col
  bands, K=128) into eps (row 32q+d = E[d] of band-q batches) -> copy
  PSUM->SBUF bf16 -> DMA out; host unpermutes E'[i,32q+d,c] ->
  E[i*4F+q*F+c, d].  ACT/DVE roles alternate per block to balance load;
  PSUM: ug012 x2 (6 banks) + ug3 (1) + eps (1) = 8.
"""

import os

import ml_dtypes
import numpy as np

import concourse.bacc as bacc
import concourse.tile as tile
from concourse import mybir
from concourse import bass_utils

B_TOTAL, D, H = 1048576, 32, 4
NCORES = 8
R = B_TOTAL // NCORES  # rows per core
BF16 = ml_dtypes.bfloat16

F = int(os.environ.get("NNK_F", "4096"))  # main megatile free size (cols)
OUT_DT = os.environ.get("NNK_OUT", "bf16")  # bf16 | f32 output E dtype
WARM = int(os.environ.get("NNK_WARM", "5"))  # p-state warmup matmuls

# per-megatile free sizes; 4*sum(MTS) == R (rows per core).  Ramp-up and
# ramp-down ladders keep the pipeline fill (first loads) and drain (last
# stores) short while the middle uses big tiles.
_head, _tail = [1024, 2048], [2048, 2048, 1024]
_mid_total = R // 4 - sum(_head) - sum(_tail)
_mid = [F] * (_mid_total // F)
if _mid_total % F:
    _mid.append(_mid_total % F)
MTS = _head + _mid + _tail
assert 4 * sum(MTS) == R and all(m % 512 == 0 and m > 0 for m in MTS)
TOTC = sum(MTS)  # total columns per core

_module_cache = {}


def _build_module(rows, b2_zero):
    key = (rows, b2_zero, tuple(MTS), OUT_DT, WARM)
    if key in _module_cache:
        return _module_cache[key]

    f32 = mybir.dt.float32
    bf = mybir.dt.bfloat16
    odt = bf if OUT_DT == "bf16" else f32

    nc = bacc.Bacc("TRN2", target_bir_lowering=False, debug=False,
                   num_devices=NCORES)

    nmt = len(MTS)
    fmax = max(MTS)

    ZT = nc.dram_tensor("ZT", (128, TOTC), bf, kind="ExternalInput").ap()
    A4 = nc.dram_tensor("A4", (128, 128), bf, kind="ExternalInput").ap()
    W2S = nc.dram_tensor("W2S", (128, D), bf, kind="ExternalInput").ap()
    B1V = nc.dram_tensor("B1V", (128, 1), f32, kind="ExternalInput").ap()
    B2V = nc.dram_tensor("B2V", (128, 1), f32, kind="ExternalInput").ap()
    EP = nc.dram_tensor("EP", (128, TOTC), odt, kind="ExternalOutput").ap()

    def relu_op(eng, vg, ug, b1v):
        if eng == "A":
            nc.scalar.activation(vg, ug, mybir.ActivationFunctionType.Relu,
                                 bias=b1v, scale=1.0)
        else:
            nc.vector.tensor_scalar(vg, ug, b1v, 0.0,
                                    mybir.AluOpType.add, mybir.AluOpType.max)

    def copy_op(eng, out, in_, b2v):
        if not b2_zero or eng == "A":
            nc.scalar.activation(out, in_,
                                 mybir.ActivationFunctionType.Identity,
                                 bias=(0.0 if b2_zero else b2v), scale=1.0)
        else:
            nc.vector.tensor_copy(out, in_)

    with tile.TileContext(nc) as tc:
        with (
            tc.tile_pool(name="const", bufs=1) as constp,
            tc.tile_pool(name="zt", bufs=2) as ztp,
            tc.tile_pool(name="vv", bufs=2) as vp,
            tc.tile_pool(name="en", bufs=2) as enp,
            tc.tile_pool(name="pu", bufs=1, space="PSUM") as pup,
            tc.tile_pool(name="pe", bufs=1, space="PSUM") as pep,
        ):
            # DMA order matters on the single input ring: A4 (needed by the
            # first mm1) and the first Z tile go first; the remaining
            # consts are only needed once relu/mm3 start.
            acst = constp.tile([128, 128], bf, name="cA4")
            nc.sync.dma_start(out=acst, in_=A4)
            fmax = max(MTS)
            zt0 = ztp.tile([128, fmax], bf, tag="zt")
            # first 512 cols (all block 0 needs) on the SP ring right after
            # A4; the rest rides the idle Activation ring in parallel.  The
            # dispatches have no upstream deps, so no head-of-line risk.
            nc.sync.dma_start(out=zt0[:, :512], in_=ZT[:, :512])
            nc.scalar.dma_start(out=zt0[:, 512:MTS[0]],
                                in_=ZT[:, 512:MTS[0]])
            wcst = constp.tile([128, D], bf, name="cW2S")
            nc.sync.dma_start(out=wcst, in_=W2S)
            b1v = constp.tile([128, 1], f32)
            nc.sync.dma_start(out=b1v, in_=B1V)
            b2v = constp.tile([128, 1], f32)
            nc.sync.dma_start(out=b2v, in_=B2V)

            if WARM:
                # dummy matmuls during the initial DMA wait: keeps the PE
                # busy so the p-state ramp to full clock completes before
                # real work arrives.  Reads a memset scratch tile (no DMA
                # dependency); writes land in the eps bank and are
                # overwritten by block 0.
                wtile = constp.tile([128, 512], bf, name="warm")
                nc.gpsimd.memset(wtile, 0.0)
                epw = pep.tile([128, 512], f32, bufs=1, tag="eps",
                               name="epw")
                for _ in range(WARM):
                    nc.tensor.matmul(epw, lhsT=wtile[:, 0:128], rhs=wtile,
                                     start=True, stop=True,
                                     tile_position=(0, 0))

            off = 0
            for i, fi in enumerate(MTS):
                if i == 0:
                    zt = zt0
                else:
                    zt = ztp.tile([128, fmax], bf, tag="zt")
                    nc.sync.dma_start(out=zt[:, :fi],
                                      in_=ZT[:, off:off + fi])
                enat = enp.tile([128, fmax], odt, tag="en")
                for t in range(fi // 512):
                    sl = slice(512 * t, 512 * (t + 1))
                    # PSUM banks: ug012 (3) x2 bufs + ug3 (1) + eps (1) = 8.
                    ug012 = pup.tile([128, 1536], f32, tag="ug012",
                                     name="ug012", bufs=2)
                    ug3 = pup.tile([128, 512], f32, tag="ug3", name="ug3",
                                   bufs=1)
                    for q in range(4):
                        nc.tensor.matmul(
                            ug012[:, 512 * q:512 * (q + 1)] if q < 3 else ug3,
                            lhsT=acst[32 * q:32 * (q + 1), :],
                            rhs=zt[32 * q:32 * (q + 1), sl],
                            start=True, stop=True,
                            tile_position=(32 * q, 0),
                        )
                    # alternate ACT/DVE roles per block: one engine does the
                    # wide relu012, the other does relu3 (which gates the
                    # single-buffered ug3) + the eps copy.  Alternation keeps
                    # consecutive blocks' critical relus on different
                    # engines (no head-of-line blocking).
                    e_wide, e_rest = ("A", "D") if t % 2 == 0 else ("D", "A")
                    vg012 = vp.tile([128, 1536], bf, tag="vg012",
                                    name="vg012")
                    vg3 = vp.tile([128, 512], bf, tag="vg3", name="vg3")
                    relu_op(e_rest, vg3, ug3, b1v)
                    relu_op(e_wide, vg012, ug012, b1v)
                    eps = pep.tile([128, 512], f32, bufs=1, tag="eps")
                    for q in range(4):
                        nc.tensor.matmul(
                            eps[32 * q:32 * (q + 1), :],
                            lhsT=wcst,
                            rhs=vg012[:, 512 * q:512 * (q + 1)]
                            if q < 3 else vg3,
                            start=True, stop=True,
                            tile_position=(0, 32 * q),
                        )
                    copy_op(e_rest, enat[:, sl], eps, b2v)
                    if i >= nmt - 2:
                        # chunked trailing stores so the final DMAs drain
                        # early instead of queueing a late 1MB transfer
                        osl = slice(off + 512 * t, off + 512 * (t + 1))
                        nc.sync.dma_start(out=EP[:, osl], in_=enat[:, sl])
                if i < nmt - 2:
                    nc.sync.dma_start(out=EP[:, off:off + fi],
                                      in_=enat[:, :fi])
                off += fi

    nc.compile()
    _module_cache[key] = nc
    return nc


def _fold_params(matrix, W1, b1, W2, b2):
    """Host-side fold of the tiny params into A4/W2S/B1V/B2V (a few KB)."""
    matrix = np.asarray(matrix, np.float32)
    W1 = np.asarray(W1, np.float32)
    b1 = np.asarray(b1, np.float32)
    W2 = np.asarray(W2, np.float32)
    b2 = np.asarray(b2, np.float32)

    alpha_est = matrix * (matrix > np.float32(0.1)).astype(np.float32)
    mask = (alpha_est > np.float32(0.1)).astype(np.float32)  # (D, D)
    cnt = mask.sum(axis=1)  # (D,)
    scale = np.where(cnt > 0, np.float32(1.0) / np.maximum(cnt, 1.0),
                     np.float32(0.0)).astype(np.float32)
    M2 = (mask.T * scale[None, :]).astype(np.float32)  # M2[j,d]

    A = np.zeros((D, D * H), np.float32)
    for h in range(H):
        Ah = M2 * W1[None, :, 1, h]  # (j, d): M2[j,d] * W1[d,1,h]
        Ah[np.arange(D), np.arange(D)] += W1[:, 0, h]
        A[:, D * h:D * (h + 1)] = Ah
    A4 = np.ascontiguousarray(np.tile(A, (4, 1)))  # (128, 128)

    W2S = np.zeros((D * H, D), np.float32)
    W2S[np.arange(D * H), np.tile(np.arange(D), H)] = W2.T.reshape(-1)
    B1V = np.ascontiguousarray(b1.T.reshape(D * H, 1))
    B2V = np.ascontiguousarray(np.tile(b2, H).reshape(D * H, 1))
    b2_zero = not np.any(b2)
    return A4, W2S, B1V, B2V, b2_zero


def _prep_z(Z):
    """Per megatile i (free size fi at col offset oi):
    Zt[c][32q+j, oi+col] = Z[c*R + 4*oi + q*fi + col, j] in bf16."""
    Zr = np.asarray(Z, np.float32).astype(BF16).reshape(NCORES, R, D)
    out = np.empty((NCORES, 128, TOTC), BF16)
    for c in range(NCORES):
        off = 0
        for fi in MTS:
            blk = Zr[c, 4 * off:4 * (off + fi)]    # (4*fi, D)
            blk = blk.reshape(4, fi, D).transpose(0, 2, 1)  # q, j, col
            out[c, :, off:off + fi] = blk.reshape(128, fi)
            off += fi
    return out


def _unpermute_e(chunks):
    """Invert: EP[32q+d, oi+col] = E[4*oi + q*fi + col, d] per core."""
    es = []
    for ep in chunks:
        e = np.empty((R, D), ep.dtype)
        off = 0
        for fi in MTS:
            blk = ep[:, off:off + fi].reshape(4, D, fi)
            e[4 * off:4 * (off + fi)] = (
                blk.transpose(0, 2, 1).reshape(4 * fi, D))
            off += fi
        es.append(e)
    out = np.concatenate(es, axis=0)
    if out.dtype != np.float32:
        out = out.astype(np.float32)
    return out


def _run(Z, matrix, W1, b1, W2, b2, trace=False):
    assert np.asarray(Z).shape == (B_TOTAL, D)
    A4, W2S, B1V, B2V, b2_zero = _fold_params(matrix, W1, b1, W2, b2)
    nc = _build_module(R, b2_zero)

    Zt = _prep_z(Z)
    cst = {
        "A4": np.ascontiguousarray(A4.astype(BF16)),
        "W2S": np.ascontiguousarray(W2S.astype(BF16)),
        "B1V": B1V, "B2V": B2V,
    }
    in_maps = [{**cst, "ZT": np.ascontiguousarray(Zt[c])}
               for c in range(NCORES)]
    res = bass_utils.run_bass_kernel_spmd(
        nc, in_maps, core_ids=list(range(NCORES)), trace=trace)
    out = _unpermute_e([r["EP"] for r in res.results])
    return out, res


def kernel(Z, matrix, W1, b1, W2, b2):
    out, _ = _run(Z, matrix, W1, b1, W2, b2, trace=False)
    return out
